# revision 1
# baseline (speedup 1.0000x reference)
"""BRISQUE kernel for Trainium2 (8 NeuronCores, data-parallel over batch).

Full NSS pipeline on device. Per core: 2 images.
Engine split per 128-row band:
  PE   : luma (diag matmuls, fp32r), H-direction gaussian conv (Toeplitz fp16),
         d = y - mu (accumulated onto the mu PSUM tile), both resize stages.
  DVE  : W-direction gaussian conv (7-tap fp16 MACs, alignment kept via an
         odd-shifted copy), t2/|t2|/reciprocal/xn, shifted products + counts.
  ACT  : squares, sqrt, sign, copies PSUM->SBUF, with accum_out row-sums.
  DMA  : input bands, row/col shifted copies of xn (wraparound built in).
Host: tiny per-image feature finalization (argmin over gamma table) + RBF SVR.
"""

import math
import sys

if "/opt/trn_rl_repo" not in sys.path:
    sys.path.insert(0, "/opt/trn_rl_repo")

import numpy as np

import concourse.bass as bass
import concourse.mybir as mybir
import concourse.tile as tile
from concourse.bass_utils import run_bass_kernel_spmd

f32 = mybir.dt.float32
f16 = mybir.dt.float16
f32r = mybir.dt.float32r
Alu = mybir.AluOpType
Act = mybir.ActivationFunctionType

N_CORES = 8
FULL_N, C, FULL_H, FULL_W = 16, 3, 1080, 1920
IMGS = FULL_N // N_CORES
KS, KSIG = 7, 7.0 / 6.0
WTAPS_PE = (0, 5, 6)   # W-conv taps computed on PE (2D-conv matmuls)
WTAPS_DVE = (2, 4, 1, 3)  # first entry must be even (TS first tap)
PB = 128          # partition band
NCK = 512         # psum chunk = one PSUM bank (512 fp32)
NSTATS = 22       # per image-scale stat streams
STATS_PAD = 32

LUMA_W = (0.299, 0.587, 0.114)

FEATURE_RANGES = np.asarray(
    [[0.338, 10], [0.017204, 0.806612], [0.236, 1.642], [-0.123884, 0.20293],
     [0.000155, 0.712298], [0.001122, 0.470257], [0.244, 1.641],
     [-0.123586, 0.179083], [0.000152, 0.710456], [0.000975, 0.470984],
     [0.249, 1.555], [-0.135687, 0.100858], [0.000174, 0.684173],
     [0.000913, 0.534174], [0.258, 1.561], [-0.143408, 0.100486],
     [0.000179, 0.685696], [0.000888, 0.536508], [0.471, 3.264],
     [0.012809, 0.703171], [0.218, 1.046], [-0.094876, 0.187459],
     [1.5e-05, 0.442057], [0.001272, 0.40803], [0.222, 1.042],
     [-0.115772, 0.162604], [1.6e-05, 0.444362], [0.001374, 0.40243],
     [0.227, 0.996], [-0.117188, 0.098323], [3e-05, 0.531903],
     [0.001122, 0.369589], [0.228, 0.99], [-0.12243, 0.098658],
     [2.8e-05, 0.530092], [0.001118, 0.370399]], dtype=np.float32)


# ---------------------------------------------------------------------------
# host-side constant construction
# ---------------------------------------------------------------------------

def _gauss_taps():
    ax = np.arange(KS, dtype=np.float64) - (KS - 1) / 2
    g = np.exp(-(ax ** 2) / (2 * KSIG ** 2))
    k2 = np.outer(g, g)
    return g * (g.sum() / k2.sum())  # == g / g.sum()


def _cubic(x):
    ax = np.abs(x); ax2 = ax * ax; ax3 = ax2 * ax
    return ((1.5 * ax3 - 2.5 * ax2 + 1) * (ax <= 1)
            + (-0.5 * ax3 + 2.5 * ax2 - 4 * ax + 2) * ((ax > 1) & (ax <= 2)))


def _resize_matrix(in_len, scale=0.5):
    out_len = int(np.ceil(in_len * scale))
    kw = 4.0 / scale
    u = (np.arange(1, out_len + 1, dtype=np.float64) / scale) + 0.5 * (1 - 1 / scale)
    left = np.floor(u - kw / 2)
    P = int(np.ceil(kw)) + 2
    ind = left[:, None] + np.arange(P)[None, :]
    wgt = scale * _cubic((u[:, None] - ind) * scale)
    wgt = wgt / wgt.sum(1, keepdims=True)
    aux = np.concatenate([np.arange(in_len), np.arange(in_len)[::-1]])
    idx = aux[np.mod(ind.astype(np.int64) - 1, 2 * in_len)]
    M = np.zeros((out_len, in_len), np.float64)
    np.add.at(M, (np.repeat(np.arange(out_len), P), idx.ravel()), wgt.ravel())
    return M, wgt[0]


def _bands(H):
    out = []
    h0 = 0
    while h0 < H:
        out.append((h0, min(PB, H - h0)))
        h0 += PB
    return out


def _chunks(W):
    out = []
    c0 = 0
    while c0 < W:
        out.append((c0, min(NCK, W - c0)))
        c0 += NCK
    return out


def build_consts(H, W):
    """Device constant arrays, replicated to every core."""
    g = _gauss_taps()
    consts = {}

    # H-conv Toeplitz blocks: T_d[k, m] = g[k - m + 3 + 128*d], d in {-1,0,1}
    # mu[m] = sum_t g[t] x[m + t - 3]; src k = m + t - 3 - 128d
    # => T_d[k, m] = g[k - m + 3 + 128 d]
    toep = np.zeros((3, PB, PB), np.float64)
    for di, d in enumerate((-1, 0, 1)):
        for k in range(PB):
            for m in range(PB):
                t = k - m + 3 + 128 * d
                if 0 <= t < KS:
                    toep[di, k, m] = g[t]
    consts["TOEP"] = toep.astype(np.float16)
    # PE-handled W-taps: blocks g[s] * T_d for s in WTAPS_PE, d in (-1,0,1)
    tw = []
    for s in WTAPS_PE:
        for di in range(3):
            tw.append(g[s] * toep[di])
    consts["TOEPW"] = np.stack(tw).astype(np.float16)

    # luma diagonals (y_tilde = y/2 = 255/2 * sum_c w_c x_c)
    diag = np.zeros((3, PB, PB), np.float32)
    for c in range(3):
        np.fill_diagonal(diag[c], LUMA_W[c] * 255.0 / 2.0)
    consts["DIAG"] = diag.astype(np.float16)

    # H-resize blocks (exact slices of Mh, transposed for lhsT)
    Mh, _ = _resize_matrix(H)
    H2 = Mh.shape[0]
    b_in, b_out = _bands(H), _bands(H2)
    rh_blocks = []     # list of arrays
    rh_sched = []      # per out band: list of (in_band, block_idx)
    for o, (oh0, op_) in enumerate(b_out):
        lst = []
        for ib, (ih0, ip_) in enumerate(b_in):
            blk = Mh[oh0:oh0 + op_, ih0:ih0 + ip_]
            if np.abs(blk).max() > 0:
                arr = np.zeros((PB, PB), np.float16)
                arr[:ip_, :op_] = blk.T.astype(np.float16)
                lst.append((ib, len(rh_blocks)))
                rh_blocks.append(arr)
        rh_sched.append(lst)
    consts["RH"] = np.stack(rh_blocks)

    # W-resize taps: out[p] = sum_t wt[t] * in[2p - 3 + t], mirror-guarded
    Mw, wrow = _resize_matrix(W)
    wt8 = wrow[1:9]
    iw = np.zeros((8, PB, PB), np.float32)
    for t in range(8):
        np.fill_diagonal(iw[t], wt8[t])
    consts["IW"] = iw.astype(np.float16)
    consts["IDENT"] = np.eye(PB).astype(np.float16)

    meta = {"H2": H2, "W2": Mw.shape[0], "rh_sched": rh_sched}
    return consts, meta


# ---------------------------------------------------------------------------
# walrus single-wait fixup (from the working baseline)
# ---------------------------------------------------------------------------

def _split_multiwait_drains(nc):
    fixn = 0
    for f in nc.m.functions:
        for bb in f.blocks:
            out = []
            changed = False
            for inst in bb.instructions:
                si = inst.sync_info
                if si is not None and len(si.on_wait) > 1:
                    waits = list(si.on_wait)
                    for wv in waits[:-1]:
                        fixn += 1
                        nop = mybir.InstNoOp(
                            name=f"I-waitfix-{fixn}", ins=[], outs=[])
                        nop.sync_info = mybir.SyncInfo(
                            on_wait=[wv], on_update=[])
                        nop.engine = inst.engine
                        out.append(nop)
                    inst.sync_info = mybir.SyncInfo(
                        on_wait=[waits[-1]], on_update=list(si.on_update))
                    changed = True
                out.append(inst)
            if changed:
                bb.instructions = out
    return fixn


# ---------------------------------------------------------------------------
# device program
# ---------------------------------------------------------------------------

_NC_CACHE = {}


def _stt(nc, out, in0, scalar, in1, op0, op1, accum_out=None):
    nc.vector.scalar_tensor_tensor(
        out=out, in0=in0, scalar=scalar, in1=in1, op0=op0, op1=op1,
        accum_out=accum_out)


def _emit_wconv(nc, out, src, src1, p, W, g):
    """out[:p, 0:W] = DVE-subset taps of the 7-tap zero-pad conv.

    src: (128, W+6) fp16, data at cols [3, W+3), zero pads elsewhere.
    src1: (128, W+5) fp16 = src shifted left 1, for odd taps (alignment).
    PE adds the WTAPS_PE taps directly into the H-conv PSUM group.
    """
    s0 = WTAPS_DVE[0]
    nc.vector.tensor_scalar(
        out=out[:p], in0=src[:p, s0:s0 + W], scalar1=float(g[s0]), scalar2=None,
        op0=Alu.mult)
    for s in WTAPS_DVE[1:]:
        in0 = src[:p, s:s + W] if s % 2 == 0 else src1[:p, s - 1:s - 1 + W]
        _stt(nc, out[:p], in0, float(g[s]), out[:p], Alu.mult, Alu.add)


def _build_nc(H, W):
    key = (H, W)
    if key in _NC_CACHE:
        return _NC_CACHE[key]

    consts, meta = build_consts(H, W)
    H2, W2 = meta["H2"], meta["W2"]
    rh_sched = meta["rh_sched"]
    g = _gauss_taps()

    nc = bass.Bass()
    xs = nc.dram_tensor("xs", [IMGS, C, H, W], f16, kind="ExternalInput")
    d_toep = nc.dram_tensor("TOEP", list(consts["TOEP"].shape), f16, kind="ExternalInput")
    d_toepw = nc.dram_tensor("TOEPW", list(consts["TOEPW"].shape), f16, kind="ExternalInput")
    d_id = nc.dram_tensor("IDENT", [PB, PB], f16, kind="ExternalInput")
    d_diag = nc.dram_tensor("DIAG", [3, PB, PB], f16, kind="ExternalInput")
    d_rh = nc.dram_tensor("RH", list(consts["RH"].shape), f16, kind="ExternalInput")
    d_iw = nc.dram_tensor("IW", [8, PB, PB], f16, kind="ExternalInput")
    so = nc.dram_tensor("stats", [IMGS, 2, PB, STATS_PAD], f32, kind="ExternalOutput")

    bands1, bands2 = _bands(H), _bands(H2)
    cks1, cks2 = _chunks(W), _chunks(W2)
    NB1, NB2 = len(bands1), len(bands2)

    with tile.TileContext(nc) as tc:
        pools = {}

        def pool(name, **kw):
            if name not in pools:
                pools[name] = tc.alloc_tile_pool(
                    name=name, bufs=kw.get("bufs", 2),
                    space=kw.get("space", "SBUF"))
            return pools[name]

        csts = pool("consts", bufs=1)
        ps = pool("psum", bufs=2, space="PSUM")

        # ---- load constants once (stacks of (128,128) as wide 2D tiles) ----
        def load_stack(name, dram, nblk, dt_):
            t = csts.tile([PB, nblk * PB], dt_, tag=name)
            for i in range(nblk):
                nc.sync.dma_start(out=t[:, i * PB:(i + 1) * PB], in_=dram[i])
            return t

        toep_t = load_stack("toep", d_toep, 3, f16)
        ntw = consts["TOEPW"].shape[0]
        toepw_t = load_stack("toepw", d_toepw, ntw, f16)
        diag_t = load_stack("diag", d_diag, 3, f16)
        nrh = consts["RH"].shape[0]
        rh_t = load_stack("rh", d_rh, nrh, f16)
        iw_t = load_stack("iw", d_iw, 8, f16)
        eps_t = csts.tile([PB, 1], f32, tag="eps")
        nc.vector.memset(eps_t, 1e-8)
        one1_t = csts.tile([PB, 1], f32, tag="one1")
        nc.vector.memset(one1_t, 1.0)
        half_t = csts.tile([PB, 1], f32, tag="half")
        nc.vector.memset(half_t, 0.5)
        d_ident = None  # identity built below via dram const
        ident_t = csts.tile([PB, PB], f16, tag="ident")
        nc.sync.dma_start(out=ident_t[:, :], in_=d_id[:, :])

        def nss_scale(img, sidx, bandsS, cksS, WS, zt_tiles, sq_tiles, feed_band):
            """Emit NSS for one image-scale.

            feed_band(b): emits front-end producing zt_tiles[b], sq_tiles[b]
            (padded fp16 tiles, data cols [3, WS+3)).
            """
            NB = len(bandsS)
            npfx = f"i{img}s{sidx}"

            acc = {}
            for nm in ("xx", "ax"):
                a = pool("acc", bufs=1).tile([PB, NB1 + 2], f32, tag=f"acc_{nm}")
                nc.vector.memset(a, 0.0)
                acc[nm] = a

            wy_tiles, wsq_tiles, xn_tiles = [None] * NB, [None] * NB, [None] * NB

            def front_and_wconv(b):
                feed_band(b)
                h0, p = bandsS[b]
                zt, sqt = zt_tiles[b], sq_tiles[b]
                # odd-shifted copies (ACT) for alignment
                zt1 = pool("zt1", bufs=1).tile([PB, WS + 5], f16, tag="zt1")
                nc.vector.tensor_copy(out=zt1[:p], in_=zt[:p, 1:WS + 6])
                sq1 = pool("sq1", bufs=1).tile([PB, WS + 5], f16, tag="sq1")
                nc.vector.tensor_copy(out=sq1[:p], in_=sqt[:p, 1:WS + 6])
                wy = pool("wy", bufs=3).tile([PB, WS], f16, tag="wy")
                _emit_wconv(nc, wy, zt, zt1, p, WS, g)
                wsq = pool("wsq", bufs=3).tile([PB, WS], f16, tag="wsq")
                _emit_wconv(nc, wsq, sqt, sq1, p, WS, g)
                wy_tiles[b], wsq_tiles[b] = wy, wsq

            def nss_band(b):
                h0, p = bandsS[b]
                zt = zt_tiles[b]
                # --- H-conv + PE W-taps: mu and s2c ---
                mu_ps = ps.tile([PB, WS], f32, tag="big")
                s2_ps = ps.tile([PB, WS], f32, tag="big")
                deltas = [d for d in (-1, 0, 1) if 0 <= b + d < NB]
                for dst_ps, w_tiles, pad_tiles in (
                        (mu_ps, wy_tiles, zt_tiles), (s2_ps, wsq_tiles, sq_tiles)):
                    for ci, (c0, cw) in enumerate(cksS):
                        nmm = len(deltas) * (1 + len(WTAPS_PE))
                        k = 0
                        for di, d in enumerate(deltas):
                            sp = bandsS[b + d][1]  # src rows
                            nc.tensor.matmul(
                                dst_ps[:p, c0:c0 + cw],
                                toep_t[:sp, (d + 1) * PB:(d + 1) * PB + p],
                                w_tiles[b + d][:sp, c0:c0 + cw],
                                start=(k == 0), stop=(k == nmm - 1))
                            k += 1
                            for si, s in enumerate(WTAPS_PE):
                                blk = (si * 3 + (d + 1)) * PB
                                nc.tensor.matmul(
                                    dst_ps[:p, c0:c0 + cw],
                                    toepw_t[:sp, blk:blk + p],
                                    pad_tiles[b + d][:sp, c0 + s:c0 + s + cw],
                                    start=(k == 0), stop=(k == nmm - 1))
                                k += 1
                # --- sigma / reciprocal ---
                musq = pool("f32scr", bufs=3).tile([PB, WS], f32, tag="f32scr")
                nc.scalar.activation(out=musq[:p], in_=mu_ps[:p], func=Act.Square)
                t2 = pool("f32scr", bufs=3).tile([PB, WS], f32, tag="f32scr")
                _stt(nc, t2[:p], s2_ps[:p], 1.0, musq[:p], Alu.mult, Alu.subtract)
                dmt = pool("f32scr", bufs=3).tile([PB, WS], f32, tag="f32scr")
                _stt(nc, dmt[:p], mu_ps[:p], -1.0, zt[:p, 3:3 + WS],
                     Alu.mult, Alu.add)
                t2a = pool("f32scr", bufs=3).tile([PB, WS], f32, tag="f32scr")
                nc.scalar.activation(out=t2a[:p], in_=t2[:p], func=Act.Abs)
                sig = pool("f32scr", bufs=3).tile([PB, WS], f32, tag="f32scr")
                nc.scalar.activation(out=sig[:p], in_=t2a[:p], func=Act.Sqrt,
                                     bias=eps_t[:p], scale=4.0)
                lnd = pool("f32scr", bufs=3).tile([PB, WS], f32, tag="f32scr")
                nc.scalar.activation(out=lnd[:p], in_=sig[:p], func=Act.Ln,
                                     bias=half_t[:p], scale=0.5)
                rcp = pool("f32scr", bufs=3).tile([PB, WS], f32, tag="f32scr")
                nc.scalar.activation(out=rcp[:p], in_=lnd[:p], func=Act.Exp,
                                     bias=0.0, scale=-1.0)
                # --- xn = (z - mu) * (2/(1+sigma)) ---
                xn = pool("xn", bufs=1).tile([PB, WS], f16, tag=f"xn_{b}")
                _stt(nc, xn[:p], dmt[:p], 1.0, rcp[:p], Alu.mult, Alu.mult)
                xn_tiles[b] = xn
                # --- GGD accums ---
                scr = pool("xc1").tile([PB, WS], f16, tag="xc1")
                nc.scalar.activation(out=scr[:p], in_=xn[:p], func=Act.Square,
                                     accum_out=acc["xx"][:p, b:b + 1])
                scr2 = pool("xdn").tile([PB, WS], f16, tag="xdn")
                nc.scalar.activation(out=scr2[:p], in_=xn[:p], func=Act.Abs,
                                     accum_out=acc["ax"][:p, b:b + 1])

            # software-pipelined emission
            front_and_wconv(0)
            for b in range(NB):
                if b + 1 < NB:
                    front_and_wconv(b + 1)
                nss_band(b)

            # ---- products via PE Gram diagonals ----
            # fields per band: q=xn^2, r=xn*|xn|, a=|xn|, s=sign(xn), width
            # WS+1 with col 0 = wraparound copy of the last data column.
            # All four roll shifts reduce to Grams of (f, fup):
            #   (0,1): f x f-col      (1,0): fup x f
            #   (1,1): fup x f-col    (-1,1): f x fup-col
            # diag-sums extracted with one identity-masked STT+accum each.
            FIELDS = ("q", "r", "a", "s")
            FSTREAM = {"q": "pp", "r": "pap", "a": "abs", "s": "sgn"}
            ncks = (WS + 127) // PB
            gram = ps.tile([PB, 16 * PB], f32, tag="big")
            f_tiles = {}
            fup_tiles = {}
            row0 = {}

            def mk_fields(b):
                h0, p = bandsS[b]
                xn = xn_tiles[b]
                ft = {}
                ftags = ("xc1", "xdn", "xup", "wy")
                for fi, fn in enumerate(FIELDS):
                    t = pool(ftags[fi], bufs=2).tile([PB, WS + 1], f16,
                                                     tag=ftags[fi])
                    ft[fn] = t
                af = ft["a"]
                nc.scalar.activation(out=ft["q"][:p, 1:WS + 1], in_=xn[:p],
                                     func=Act.Square,
                                     accum_out=acc["xx"][:p, b:b + 1])
                nc.scalar.activation(out=af[:p, 1:WS + 1], in_=xn[:p],
                                     func=Act.Abs,
                                     accum_out=acc["ax"][:p, b:b + 1])
                nc.scalar.activation(out=ft["s"][:p, 1:WS + 1], in_=xn[:p],
                                     func=Act.Sign)
                _stt(nc, ft["r"][:p, 1:WS + 1], xn[:p], 1.0,
                     af[:p, 1:WS + 1], Alu.mult, Alu.mult)
                for fi, fn in enumerate(FIELDS):
                    nc.sync.dma_start(out=ft[fn][:p, 0:1],
                                      in_=ft[fn][:p, WS:WS + 1])
                f_tiles[b] = ft
                if b == 0:
                    r0 = pool("row0", bufs=1).tile([4, WS + 1], f16, tag="row0")
                    for fi, fn in enumerate(FIELDS):
                        nc.sync.dma_start(out=r0[fi:fi + 1], in_=ft[fn][0:1])
                    row0["t"] = r0


            def mk_fup(b):
                h0, p = bandsS[b]
                ft = f_tiles[b]
                fu = {}
                utags = ("wsq", "zt1", "sq1", "sqt")
                for fi, fn in enumerate(FIELDS):
                    t = pool(utags[fi], bufs=2).tile([PB, WS + 1], f16,
                                                     tag=utags[fi])
                    eng = nc.gpsimd if fi % 2 else nc.sync
                    if p > 1:
                        eng.dma_start(out=t[0:p - 1], in_=ft[fn][1:p])
                    if b + 1 < NB:
                        nxt = f_tiles[b + 1][fn]
                        nc.sync.dma_start(out=t[p - 1:p], in_=nxt[0:1])
                    else:
                        nc.sync.dma_start(out=t[p - 1:p],
                                          in_=row0["t"][fi:fi + 1])
                    fu[fn] = t
                fup_tiles[b] = fu

            def grams(b):
                h0, p = bandsS[b]
                ft, fu = f_tiles[b], fup_tiles[b]
                for fi, fn in enumerate(FIELDS):
                    f, up = ft[fn], fu[fn]
                    for ci in range(ncks):
                        c = ci * PB
                        m = min(PB, WS - c)
                        first = (b == 0 and ci == 0)
                        last = (b == NB - 1 and ci == ncks - 1)
                        pairs = (  # (shift si, lhsT, rhs)
                            (0, f[:p, 1 + c:1 + c + m], f[:p, c:c + m]),
                            (1, up[:p, 1 + c:1 + c + m], f[:p, 1 + c:1 + c + m]),
                            (2, up[:p, 1 + c:1 + c + m], f[:p, c:c + m]),
                            (3, f[:p, 1 + c:1 + c + m], up[:p, c:c + m]),
                        )
                        for si, lh, rh in pairs:
                            g0 = (fi * 4 + si) * PB
                            nc.tensor.matmul(
                                gram[:m, g0:g0 + m], lh, rh,
                                start=first, stop=last,
                                skip_group_check=True)

            mk_fields(0)
            for b in range(NB):
                if b + 1 < NB:
                    mk_fields(b + 1)
                mk_fup(b)
                grams(b)

            # ---- finalize ----
            smat = pool("smat", bufs=2).tile([PB, NSTATS], f32, tag="smat")
            nc.vector.memset(smat, 0.0)
            for k, a in enumerate([acc["xx"], acc["ax"]]):
                nc.vector.tensor_reduce(out=smat[:, k:k + 1], in_=a,
                                        axis=mybir.AxisListType.X, op=Alu.add)
            for fi, fn in enumerate(FIELDS):
                for si in range(4):
                    col = {"pp": 5, "pap": 4, "abs": 2, "sgn": 6}[FSTREAM[fn]] + 5 * si
                    g0 = (fi * 4 + si) * PB
                    dscr = pool("rgb", bufs=1).tile([PB, PB], f32, tag="rgb0")
                    _stt(nc, dscr[:, :], gram[:, g0:g0 + PB], 1.0,
                         ident_t[:, :], Alu.mult, Alu.mult,
                         accum_out=smat[:, col:col + 1])
            nc.sync.dma_start(out=so[img, sidx, :, 0:NSTATS], in_=smat[:, :])

        # ------------------------------------------------------------------
        for img in range(IMGS):
            # ---------------- scale 1 ----------------
            zt1_tiles, sq1_tiles = [None] * NB1, [None] * NB1

            def feed1(b, img=img, zt_tiles=zt1_tiles, sq_tiles=sq1_tiles):
                h0, p = bands1[b]
                rgb = []
                for c in range(3):
                    t = pool("rgb", bufs=1).tile([PB, W], f16, tag=f"rgb{c}")
                    nc.sync.dma_start(out=t[:p], in_=xs[img, c, h0:h0 + p, :])
                    rgb.append(t)
                zt = pool("ztp", bufs=1).tile([PB, W + 6], f16, tag=f"zt_{b}")
                nc.vector.memset(zt[:p, 0:3], 0.0)
                nc.vector.memset(zt[:p, W + 3:W + 6], 0.0)
                c0_, c1_, c2_ = LUMA_W
                half255 = 255.0 / 2.0
                yt = rgb[0]
                _stt(nc, yt[:p], rgb[0][:p], c0_ / c1_, rgb[1][:p],
                     Alu.mult, Alu.add)
                _stt(nc, zt[:p, 3:W + 3], yt[:p], c1_ / c2_, rgb[2][:p],
                     Alu.mult, Alu.add)
                nc.vector.tensor_scalar_mul(out=zt[:p, 3:W + 3],
                                            in0=zt[:p, 3:W + 3],
                                            scalar1=c2_ * half255)
                sqt = pool("sqt", bufs=3).tile([PB, W + 6], f16, tag="sqt")
                nc.vector.memset(sqt[:p, 0:3], 0.0)
                nc.vector.memset(sqt[:p, W + 3:W + 6], 0.0)
                nc.scalar.activation(out=sqt[:p, 3:W + 3],
                                     in_=zt[:p, 3:W + 3], func=Act.Square)
                zt_tiles[b], sq_tiles[b] = zt, sqt

            nss_scale(img, 0, bands1, cks1, W, zt1_tiles, sq1_tiles, feed1)

            # ---------------- resize ----------------
            yh1e_tiles = [None] * NB2
            for o, (oh0, op_) in enumerate(bands2):
                yh_ps = ps.tile([PB, W], f32, tag="big")
                for c0, cw in cks1:
                    blks = rh_sched[o]
                    for bi, (ib, bidx) in enumerate(blks):
                        ip_ = bands1[ib][1]
                        nc.tensor.matmul(
                            yh_ps[:op_, c0:c0 + cw],
                            rh_t[:ip_, bidx * PB:bidx * PB + op_],
                            zt1_tiles[ib][:ip_, 3 + c0:3 + c0 + cw],
                            start=(bi == 0), stop=(bi == len(blks) - 1))
                yhe = pool("yh1e", bufs=1).tile([PB, W + 16], f16,
                                                tag=f"yh1e_{o}")
                nc.scalar.copy(out=yhe[:op_, 8:8 + W], in_=yh_ps[:op_])
                # mirror guards: left = data[7..0], right = data[W-1..W-8]
                nc.sync.dma_start(out=yhe[:op_, 0:8],
                                  in_=yhe[:op_, 15:7:-1])
                nc.sync.dma_start(out=yhe[:op_, 8 + W:8 + W + 8],
                                  in_=yhe[:op_, 8 + W - 1:8 + W - 9:-1])
                yh1e_tiles[o] = yhe

            # ---------------- scale 2 ----------------
            zt2_tiles, sq2_tiles = [None] * NB2, [None] * NB2

            def feed2(b, img=img, zt_tiles=zt2_tiles, sq_tiles=sq2_tiles):
                h0, p = bands2[b]
                yhalf_ps = ps.tile([PB, W2], f32, tag="big")
                for c0, cw in cks2:
                    for t in range(8):
                        nc.tensor.matmul(
                            yhalf_ps[:p, c0:c0 + cw],
                            iw_t[:p, t * PB:t * PB + p],
                            yh1e_tiles[b][:p, 5 + t + 2 * c0: 5 + t + 2 * (c0 + cw): 2],
                            start=(t == 0), stop=(t == 7))
                zt = pool("ztp", bufs=1).tile([PB, W2 + 6], f16,
                                              tag=f"zt_{b}")
                nc.vector.memset(zt[:p, 0:3], 0.0)
                nc.vector.memset(zt[:p, W2 + 3:W2 + 6], 0.0)
                nc.scalar.copy(out=zt[:p, 3:W2 + 3], in_=yhalf_ps[:p])
                sqt = pool("sqt2", bufs=3).tile([PB, W2 + 6], f16, tag="sqt2")
                nc.vector.memset(sqt[:p, 0:3], 0.0)
                nc.vector.memset(sqt[:p, W2 + 3:W2 + 6], 0.0)
                nc.scalar.activation(out=sqt[:p, 3:W2 + 3], in_=yhalf_ps[:p],
                                     func=Act.Square)
                zt_tiles[b], sq_tiles[b] = zt, sqt

            nss_scale(img, 1, bands2, cks2, W2, zt2_tiles, sq2_tiles, feed2)

        for p in reversed(list(pools.values())):
            p.release()

    out = (nc, consts, meta)
    _NC_CACHE[key] = out
    return out


# ---------------------------------------------------------------------------
# host feature finalization
# ---------------------------------------------------------------------------

def _gammaln(v):
    v = np.asarray(v, dtype=np.float64)
    flat = v.reshape(-1)
    out = np.array([math.lgamma(t) for t in flat], dtype=np.float64)
    return out.reshape(v.shape)


_GAMMA = np.arange(0.2, 10.0 + 0.001, 0.001, dtype=np.float64).astype(np.float32)
_g64 = _GAMMA.astype(np.float64)
_R_TABLE = np.exp(2 * _gammaln(2.0 / _g64) - _gammaln(1.0 / _g64)
                  - _gammaln(3.0 / _g64)).astype(np.float32)


def _features_from_stats(stats, npix):
    """stats: (n, NSTATS) device sums for one scale -> (n, 18) features."""
    n = stats.shape[0]
    s2 = stats[:, 0] / npix
    E = stats[:, 1] / npix
    rho = s2 / np.maximum(E * E, 1e-30)
    idx = np.argmin(np.abs(rho[:, None] - _R_TABLE[None, :]), axis=-1)
    feats = [_GAMMA[idx], s2.astype(np.float32)]
    for si in range(4):
        sabs, _, pap, pp, sgn = (stats[:, 2 + 5 * si + j] for j in range(5))
        nonz = np.full_like(sabs, float(npix))
        cl = (nonz - sgn) / 2.0
        cr = (nonz + sgn) / 2.0
        ssl = np.maximum((pp - pap) / 2.0, 0.0)
        ssr = np.maximum((pp + pap) / 2.0, 0.0)
        sl = np.sqrt(ssl / np.maximum(cl, 1.0))
        sr = np.sqrt(ssr / np.maximum(cr, 1.0))
        gh = sl / np.maximum(sr, 1e-30)
        rhat = (sabs / npix) ** 2 / np.maximum(pp / npix, 1e-30)
        rhn = rhat * (gh ** 3 + 1) * (gh + 1) / (gh ** 2 + 1) ** 2
        idx = np.argmin(np.abs(_R_TABLE[None, :].astype(np.float64)
                               - rhn[:, None]), axis=-1)
        a = _GAMMA[idx]
        a64 = a.astype(np.float64)
        eta = ((sr - sl) * np.exp(
            _gammaln(2.0 / a64)
            - (_gammaln(1.0 / a64) + _gammaln(3.0 / a64)) / 2)).astype(np.float32)
        feats += [a, eta, (sl * sl).astype(np.float32), (sr * sr).astype(np.float32)]
    return np.stack(feats, -1).astype(np.float32)


def _score_from_features(f, sv, sv_coef):
    lo = FEATURE_RANGES[:, 0]
    hi = FEATURE_RANGES[:, 1]
    sf = -1.0 + 2.0 * (f - lo) / (hi - lo)
    dist = ((sf[:, :, None] - sv.T[None, :, :]) ** 2).sum(axis=1)
    kf = np.exp(np.float32(-0.05) * dist).astype(np.float32)
    return (kf @ sv_coef + np.float32(153.591)).astype(np.float32)


# ---------------------------------------------------------------------------
# entry point
# ---------------------------------------------------------------------------

_FIXED = set()


def _run_device(x, H, W):
    nc, consts, meta = _build_nc(H, W)
    if (H, W) not in _FIXED:
        _split_multiwait_drains(nc)
        _FIXED.add((H, W))
    in_maps = []
    for i in range(N_CORES):
        m = {"xs": np.ascontiguousarray(
            x[i * IMGS:(i + 1) * IMGS]).astype(np.float16)}
        for k, v in consts.items():
            m[k] = v
        in_maps.append(m)
    res = run_bass_kernel_spmd(nc, in_maps, core_ids=list(range(N_CORES)))
    stats = np.concatenate(
        [np.asarray(r["stats"], np.float64).sum(axis=2) for r in res.results],
        axis=0)
    return stats, meta, res


def kernel(x, sv, sv_coef):
    x = np.asarray(x, dtype=np.float32)
    sv = np.asarray(sv, dtype=np.float32)
    sv_coef = np.asarray(sv_coef, dtype=np.float32)
    n, _, H, W = x.shape

    stats, meta, _ = _run_device(x, H, W)
    H2, W2 = meta["H2"], meta["W2"]
    f1 = _features_from_stats(stats[:, 0, :NSTATS].astype(np.float64), H * W)
    f2 = _features_from_stats(stats[:, 1, :NSTATS].astype(np.float64), H2 * W2)
    f = np.concatenate([f1, f2], axis=-1)
    return _score_from_features(f, sv, sv_coef)

